# revision 1
# baseline (speedup 1.0000x reference)
"""Multi-head attention (B=2, N=2048, C=1024, H=16, D=64) on 8 TRN2 NeuronCores.

Sharding: core = b*4 + g  (b in {0,1} data-parallel over batch,
g in {0..3} tensor-parallel over head groups of HL=4 heads).

Per-core computation (all layouts chosen so the matmul contraction dim is
always on SBUF partitions, with no on-chip transposes; matmuls run in
float32r = TF32-like full-rate mode):
  phase 0: QT = wqT.T @ xT  -> [DL, N] (d on partitions; bias + 1/sqrt(D)
           folded in), KT likewise, V = xT.T @ wvT -> [N, DL] + ones col
  per head h, per m-chunk i (128 tokens):
    A: S^T = KT_h[:, i].T @ QT_h -> psum [128, 1024] x2 halves
       E^T = exp(S^T) via ScalarE -> short-lived SBUF tile e_i [128, 2048]
    B: O_aug^T[nb] += V_aug[i].T @ e_i[:, nb]  (4 psum accumulators [65,512]:
       rows 0..63 = O^T, row 64 = softmax denominator)
  norm: Y^T = O^T * (1/denom); reciprocal_approx_fast + K=1 ones-matmul
        partition-broadcast
  phase D: P^T = pwT.T @ Y^T -> partial projection [C, N]

Host: out[b] = sum_g P^T[b,g].T + proj_b
"""

import numpy as np
import ml_dtypes

B, N, C = 2, 2048, 1024
H = 16
D = C // H          # 64
G = 4               # head groups (tensor parallel)
HL = H // G         # 4 heads per core
DL = HL * D         # 256 local head dims
N_CORES = 8
SCALE = 1.0 / np.sqrt(np.float32(D))

MCHUNKS = N // 128  # 16

_CACHE = {}
DEBUG_TAPS = False


def build_kernel():
    import concourse.bass as bass
    import concourse.mybir as mybir
    import concourse.tile as tile
    from concourse import bacc

    f32 = mybir.dt.float32
    f32r = mybir.dt.float32r
    bf16 = mybir.dt.bfloat16

    nc = bacc.Bacc("TRN2", target_bir_lowering=False, debug=False,
                   num_devices=N_CORES)

    xt_d = nc.dram_tensor("xt", [C, N], bf16, kind="ExternalInput").ap()
    wqt_d = nc.dram_tensor("wqt", [C, DL], bf16, kind="ExternalInput").ap()
    wkt_d = nc.dram_tensor("wkt", [C, DL], bf16, kind="ExternalInput").ap()
    wvt_d = nc.dram_tensor("wvt", [C, DL], bf16, kind="ExternalInput").ap()
    bq_d = nc.dram_tensor("bq", [128, DL // 128], f32, kind="ExternalInput").ap()
    bk_d = nc.dram_tensor("bk", [128, DL // 128], f32, kind="ExternalInput").ap()
    bv_d = nc.dram_tensor("bv", [1, DL], f32r, kind="ExternalInput").ap()
    pwt_d = nc.dram_tensor("pwt", [DL, C], f32r, kind="ExternalInput").ap()
    out_d = nc.dram_tensor("out", [C, N], f32, kind="ExternalOutput").ap()
    if DEBUG_TAPS:
        dbg = {
            "dbg_qt": nc.dram_tensor("dbg_qt", [128, DL // 128, N], f32r,
                                     kind="ExternalOutput").ap(),
            "dbg_kt": nc.dram_tensor("dbg_kt", [128, DL // 128, N], f32r,
                                     kind="ExternalOutput").ap(),
            "dbg_v": nc.dram_tensor("dbg_v", [128, MCHUNKS, HL, D + 1], f32r,
                                    kind="ExternalOutput").ap(),
            "dbg_e0": nc.dram_tensor("dbg_e0", [128, N], f32r,
                                     kind="ExternalOutput").ap(),
            "dbg_ob": nc.dram_tensor("dbg_ob", [D + 1, 4, 512], f32,
                                     kind="ExternalOutput").ap(),
            "dbg_yt": nc.dram_tensor("dbg_yt", [128, DL // 128, N], f32r,
                                     kind="ExternalOutput").ap(),
            "dbg_rc": nc.dram_tensor("dbg_rc", [1, 4, 512], f32,
                                     kind="ExternalOutput").ap(),
            "dbg_bc": nc.dram_tensor("dbg_bc", [64, 4, 512], f32,
                                     kind="ExternalOutput").ap(),
        }

    CO = C // 128   # 8 chunks of the contraction dim c
    MO = DL // 128  # 2 chunks of the local head dims

    with tile.TileContext(nc) as tc:
        with (
            tc.tile_pool(name="consts", bufs=1) as consts,
            tc.tile_pool(name="acts", bufs=1) as acts,
            tc.tile_pool(name="small", bufs=4) as small,
            tc.tile_pool(name="stage", bufs=3) as stage,
            tc.tile_pool(name="psS", bufs=2, space="PSUM") as psS,
            tc.tile_pool(name="psB", bufs=4, space="PSUM") as psB,
        ):
            # ---- load weights/bias constants ----
            wq_sb = consts.tile([128, CO, DL], bf16, tag="wq")
            wk_sb = consts.tile([128, CO, DL], bf16, tag="wk")
            wv_sb = consts.tile([128, CO, DL], bf16, tag="wv")
            nc.sync.dma_start(wq_sb[:], wqt_d.rearrange("(o p) f -> p o f", p=128))
            nc.sync.dma_start(wk_sb[:], wkt_d.rearrange("(o p) f -> p o f", p=128))
            nc.sync.dma_start(wv_sb[:], wvt_d.rearrange("(o p) f -> p o f", p=128))
            pw_sb = consts.tile([128, MO, C], f32r, tag="pw")
            nc.sync.dma_start(pw_sb[:], pwt_d.rearrange("(o p) f -> p o f", p=128))
            bq_sb = consts.tile([128, MO], f32, tag="bq")
            bk_sb = consts.tile([128, MO], f32, tag="bk")
            nc.sync.dma_start(bq_sb[:], bq_d[:])
            nc.sync.dma_start(bk_sb[:], bk_d[:])
            bv_sb = consts.tile([1, DL], f32r, tag="bv")
            nc.sync.dma_start(bv_sb[:], bv_d[:])
            ones_f = consts.tile([1, 128], f32, tag="onesf")
            nc.vector.memset(ones_f[:], 1.0)
            ones_sb = consts.tile([1, 128], f32r, tag="ones")
            nc.vector.tensor_copy(ones_sb[:], ones_f[:])

            # ---- activations that stay resident ----
            qt_sb = acts.tile([128, MO, N], f32r, tag="qt")   # [DL, N]
            kt_sb = acts.tile([128, MO, N], f32r, tag="kt")   # [DL, N]
            qt2_sb = acts.tile([128, MO, N], f32r, tag="qt2")  # halves swapped
            kt2_sb = acts.tile([128, MO, N], f32r, tag="kt2")
            v_sb = acts.tile([128, MCHUNKS, HL, D + 1], f32r, tag="v")
            yt_sb = acts.tile([128, MO, N], f32r, tag="yt")   # [DL, N] normalized

            ones_col = consts.tile([128, 1], f32, tag="onescol")
            nc.vector.memset(ones_col[:], 1.0)
            nc.vector.tensor_copy(
                v_sb[:, :, :, D:],
                ones_col[:].to_broadcast([128, MCHUNKS, HL, 1]))

            # ---- phase 0 ----
            with tc.tile_pool(name="xt", bufs=1) as xt_pool:
                xt_sb = xt_pool.tile([128, CO, N], bf16, tag="xt")
                xt_r = xt_d.rearrange("(o p) n -> p o n", p=128)
                for kc in range(CO):
                    nc.sync.dma_start(xt_sb[:, kc, :], xt_r[:, kc, :])

                for w_sb, b_sb, o_sb, o2_sb in (
                        (wq_sb, bq_sb, qt_sb, qt2_sb),
                        (wk_sb, bk_sb, kt_sb, kt2_sb)):
                    for mo in range(MO):
                        for nh in range(2):
                            ps = psS.tile([128, 1024], f32, tag="pss")
                            for kc in range(CO):
                                for half in range(2):
                                    nc.tensor.matmul(
                                        ps[:, half * 512:(half + 1) * 512],
                                        lhsT=w_sb[:, kc, mo * 128:(mo + 1) * 128],
                                        rhs=xt_sb[:, kc,
                                                  nh * 1024 + half * 512:
                                                  nh * 1024 + (half + 1) * 512],
                                        start=(kc == 0), stop=(kc == CO - 1),
                                    )
                            nsl0 = slice(nh * 1024, (nh + 1) * 1024)
                            nc.vector.tensor_scalar_add(
                                o_sb[:, mo, nsl0], ps[:],
                                b_sb[:, mo:mo + 1],
                            )
                            # partition-swapped copy for A-phase row-group
                            # alternation (K=64 pairs overlap on the array)
                            nc.vector.tensor_scalar_add(
                                o2_sb[0:64, mo, nsl0], ps[64:128, :],
                                b_sb[64:128, mo:mo + 1],
                            )
                            nc.vector.tensor_scalar_add(
                                o2_sb[64:128, mo, nsl0], ps[0:64, :],
                                b_sb[0:64, mo:mo + 1],
                            )
                # V (token m on partitions) + ones-matmul bias broadcast
                for i in range(MCHUNKS):
                    ps = psS.tile([128, HL, D], f32, tag="pss")
                    for kc in range(CO):
                        nc.tensor.matmul(
                            ps[:],
                            lhsT=xt_sb[:, kc, i * 128:(i + 1) * 128],
                            rhs=wv_sb[:, kc, :],
                            start=(kc == 0), stop=False,
                        )
                    nc.tensor.matmul(
                        ps[:], lhsT=ones_sb[:], rhs=bv_sb[:],
                        start=False, stop=True,
                    )
                    nc.vector.tensor_copy(v_sb[:, i, :, :D], ps[:])

            if DEBUG_TAPS:
                nc.sync.dma_start(dbg["dbg_qt"][:], qt_sb[:])
                nc.sync.dma_start(dbg["dbg_kt"][:], kt_sb[:])
                nc.sync.dma_start(dbg["dbg_v"][:], v_sb[:])

            # ---- attention: per head, m-chunk-streamed; norm(h) is emitted
            # inside head h+1's stream so the PE never idles long enough to
            # trip the HAM clock-gate back to 1.2 GHz ----
            def emit_norm(hn, psBs_n):
                mo_n = hn // 2
                pb_n = 64 * (hn % 2)
                if DEBUG_TAPS and hn == 0:
                    for nb in range(4):
                        obf = small.tile([D + 1, 512], f32, tag="dbgob",
                                         name=f"obf{nb}")
                        nc.vector.tensor_copy(obf[:], psBs_n[nb][:])
                        nc.sync.dma_start(dbg["dbg_ob"][:, nb, :], obf[:])
                for nb in range(4):
                    nsl = slice(nb * 512, (nb + 1) * 512)
                    dn = small.tile([1, 512], f32, tag="dn")
                    nc.vector.tensor_copy(dn[:], psBs_n[nb][D:D + 1, :])
                    rc = small.tile([1, 512], f32, tag="rc")
                    nc.vector.reciprocal_approx_fast(rc[:], dn[:])
                    if DEBUG_TAPS and hn == 0:
                        nc.sync.dma_start(dbg["dbg_rc"][:, nb, :], rc[:])
                    ot = small.tile([64, 512], f32, tag="ot")
                    nc.vector.tensor_copy(ot[:], psBs_n[nb][:D, :])
                    bc = psB.tile([64, 512], f32, tag="psb",
                                  name=f"bc_{hn}_{nb}")
                    nc.tensor.matmul(bc[:], lhsT=ones_f[:, :64],
                                     rhs=rc[:], start=True, stop=True)
                    if DEBUG_TAPS and hn == 0:
                        bcf = small.tile([64, 512], f32, tag="dbgob",
                                         name=f"bcf{nb}")
                        nc.vector.tensor_copy(bcf[:], bc[:])
                        nc.sync.dma_start(dbg["dbg_bc"][:, nb, :], bcf[:])
                    nc.vector.tensor_mul(
                        yt_sb[pb_n:pb_n + D, mo_n, nsl], ot[:], bc[:])

            with (
                tc.tile_pool(name="ei", bufs=3) as ei_pool,
            ):
                # software pipeline: phase B for m-chunk i-1 is emitted
                # alongside phase A for chunk i, so the PE never stalls on
                # the exp results it just requested.
                psBs_by_h = {}
                pending = []     # queue of (h, i, ei) awaiting B matmuls

                def emit_B(hb, ib, eib):
                    if ib == 0:
                        if hb > 0:
                            emit_norm(hb - 1, psBs_by_h.pop(hb - 1))
                        psBs_by_h[hb] = [
                            psB.tile([D + 1, 512], f32, tag="psb",
                                     name=f"psb_{hb}_{nb}")
                            for nb in range(4)]
                    for nb in range(4):
                        nc.tensor.matmul(
                            psBs_by_h[hb][nb][:],
                            lhsT=v_sb[:, ib, hb, :],
                            rhs=eib[:, nb * 512:(nb + 1) * 512],
                            start=(ib == 0), stop=(ib == MCHUNKS - 1),
                        )

                for h in range(HL):
                    mo = h // 2
                    pb = 64 * (h % 2)
                    for i in range(MCHUNKS):
                        if i % 2 == 0:
                            kts, qts, pbi = kt_sb, qt_sb, pb
                        else:
                            kts, qts, pbi = kt2_sb, qt2_sb, pb ^ 64
                        ei = ei_pool.tile([128, N], f32r, tag="ei")
                        for nh in range(2):
                            ps = psS.tile([128, 1024], f32, tag="pss")
                            for half in range(2):
                                nc.tensor.matmul(
                                    ps[:, half * 512:(half + 1) * 512],
                                    lhsT=kts[pbi:pbi + D, mo,
                                             i * 128:(i + 1) * 128],
                                    rhs=qts[pbi:pbi + D, mo,
                                            nh * 1024 + half * 512:
                                            nh * 1024 + (half + 1) * 512],
                                    start=True, stop=True,
                                )
                            nc.scalar.activation(
                                ei[:, nh * 1024:(nh + 1) * 1024], ps[:],
                                mybir.ActivationFunctionType.Exp,
                            )
                        if DEBUG_TAPS and h == 0 and i == 0:
                            nc.sync.dma_start(dbg["dbg_e0"][:], ei[:])
                        if len(pending) >= 2:
                            emit_B(*pending.pop(0))
                        pending.append((h, i, ei))
                for p in pending:
                    emit_B(*p)
                emit_norm(HL - 1, psBs_by_h.pop(HL - 1))

                if DEBUG_TAPS:
                    nc.sync.dma_start(dbg["dbg_yt"][:], yt_sb[:])

                # ---- phase D: partial projection P^T = pwT.T @ Y^T ----
                for nb in range(4):
                    nsl = slice(nb * 512, (nb + 1) * 512)
                    for cc in range(8):
                        ps = psS.tile([128, 1024], f32, tag="pss")
                        for jc in range(MO):
                            nc.tensor.matmul(
                                ps[:, :512],
                                lhsT=pw_sb[:, jc, cc * 128:(cc + 1) * 128],
                                rhs=yt_sb[:, jc, nsl],
                                start=(jc == 0), stop=(jc == MO - 1),
                            )
                        st = stage.tile([128, 512], f32, tag="st")
                        nc.vector.tensor_copy(st[:], ps[:, :512])
                        nc.sync.dma_start(
                            out_d[cc * 128:(cc + 1) * 128, nsl], st[:])

    nc.compile()
    return nc


def shard_inputs(x, qkv_w, qkv_b, proj_w):
    """Build the 8 per-core input maps (host-side sharding)."""
    in_maps = []
    for core in range(N_CORES):
        b, g = divmod(core, G)
        gs = slice(g * DL, (g + 1) * DL)
        xt = np.ascontiguousarray(x[b].T)
        wq = qkv_w[0 * C:1 * C][gs] * SCALE     # fold 1/sqrt(D) into Q
        wk = qkv_w[1 * C:2 * C][gs]
        wv = qkv_w[2 * C:3 * C][gs]
        in_maps.append({
            "xt": np.ascontiguousarray(xt).astype(ml_dtypes.bfloat16),
            "wqt": np.ascontiguousarray(wq.T).astype(ml_dtypes.bfloat16),
            "wkt": np.ascontiguousarray(wk.T).astype(ml_dtypes.bfloat16),
            "wvt": np.ascontiguousarray(wv.T).astype(ml_dtypes.bfloat16),
            "bq": np.ascontiguousarray(
                (qkv_b[0 * C:1 * C][gs] * SCALE).reshape(DL // 128, 128).T),
            "bk": np.ascontiguousarray(
                qkv_b[1 * C:2 * C][gs].reshape(DL // 128, 128).T),
            "bv": np.ascontiguousarray(qkv_b[2 * C:3 * C][gs].reshape(1, DL)),
            "pwt": np.ascontiguousarray(proj_w[:, gs].T),
        })
    return in_maps


def unshard_output(results, proj_b):
    """results: list of 8 dicts with 'out' [C, N] partial projections."""
    out = np.empty((B, N, C), dtype=np.float32)
    for b in range(B):
        acc = results[b * G]["out"].astype(np.float32)
        for g in range(1, G):
            acc = acc + results[b * G + g]["out"]
        out[b] = acc.T + proj_b
    return out


def kernel(x, qkv_w, qkv_b, proj_w, proj_b):
    from concourse.bass_utils import run_bass_kernel_spmd

    x = np.asarray(x, dtype=np.float32)
    qkv_w = np.asarray(qkv_w, dtype=np.float32)
    qkv_b = np.asarray(qkv_b, dtype=np.float32)
    proj_w = np.asarray(proj_w, dtype=np.float32)
    proj_b = np.asarray(proj_b, dtype=np.float32)

    if "nc" not in _CACHE:
        _CACHE["nc"] = build_kernel()
    nc = _CACHE["nc"]

    in_maps = shard_inputs(x, qkv_w, qkv_b, proj_w)
    res = run_bass_kernel_spmd(nc, in_maps, list(range(N_CORES)))
    return unshard_output(res.results, proj_b)



# revision 3
# speedup vs baseline: 1.0479x; 1.0479x over previous
"""Multi-head attention (B=2, N=2048, C=1024, H=16, D=64) on 8 TRN2 NeuronCores.

Sharding: core = b*4 + g  (b in {0,1} data parallel over batch,
g in {0..3} tensor parallel over head groups of HL=4 heads).

v2 schedule: the ScalarE exp stream (~147us, exp runs only on ScalarE at
1 elem/lane/cycle) is the critical path; everything else hides under it:
  lead-in: xt DMA chunk-raced QK^T-mo0 matmuls -> head-0 exp starts ~16us
  head 0 loop also computes V (one psum tile per chunk); heads 1-2 compute
  QK^T-mo1 in kc-split bursts; the row-swapped copies (qt2/kt2, which let
  consecutive K=64 A-matmuls overlap on disjoint PE row groups) are
  SBUF->SBUF tensor_copies off the psum ring; per-head norm frees the four
  PSUM accumulators with single [65,512] copies, then
  reciprocal + GpSimd partition_broadcast + DVE mul, all off-band;
  projection (phase D) at the tail, f32->bf16 casts split DVE/ScalarE,
  bf16 output DMA.
Host: out[b] = sum_g P^T[b,g].T + proj_b  (bf16 partials summed in f32).
"""

import numpy as np
import ml_dtypes

B, N, C = 2, 2048, 1024
H = 16
D = C // H          # 64
G = 4               # head groups (tensor parallel)
HL = H // G         # 4 heads per core
DL = HL * D         # 256 local head dims
N_CORES = 8
SCALE = 1.0 / np.sqrt(np.float32(D))

MCHUNKS = N // 128  # 16
CO = C // 128       # 8 chunks of the contraction dim c
MO = DL // 128      # 2 chunks of the local head dims

_CACHE = {}


def build_kernel():
    import concourse.bass as bass
    import concourse.mybir as mybir
    import concourse.tile as tile
    from concourse import bacc

    f32 = mybir.dt.float32
    f32r = mybir.dt.float32r
    bf16 = mybir.dt.bfloat16

    nc = bacc.Bacc("TRN2", target_bir_lowering=False, debug=False,
                   num_devices=N_CORES)

    xt_d = nc.dram_tensor("xt", [C, N], bf16, kind="ExternalInput").ap()
    wqt_d = nc.dram_tensor("wqt", [C, DL], bf16, kind="ExternalInput").ap()
    wkt_d = nc.dram_tensor("wkt", [C, DL], bf16, kind="ExternalInput").ap()
    wvt_d = nc.dram_tensor("wvt", [C, DL], bf16, kind="ExternalInput").ap()
    bq_d = nc.dram_tensor("bq", [128, MO], f32, kind="ExternalInput").ap()
    bk_d = nc.dram_tensor("bk", [128, MO], f32, kind="ExternalInput").ap()
    bv_d = nc.dram_tensor("bv", [1, DL], f32r, kind="ExternalInput").ap()
    pwt_d = nc.dram_tensor("pwt", [DL, C], f32r, kind="ExternalInput").ap()
    out_d = nc.dram_tensor("out", [C, N], bf16, kind="ExternalOutput").ap()

    with tile.TileContext(nc) as tc:
        with (
            tc.tile_pool(name="consts", bufs=1) as consts,
            tc.tile_pool(name="acts", bufs=1) as acts,
            tc.tile_pool(name="xtp", bufs=1) as xtp,
            tc.tile_pool(name="small", bufs=4) as small,
            tc.tile_pool(name="stp", bufs=3) as stp,
            tc.tile_pool(name="ei", bufs=3) as ei_pool,
            tc.tile_pool(name="psS", bufs=2, space="PSUM") as psS,
            tc.tile_pool(name="psB", bufs=4, space="PSUM") as psB,
        ):
            # ---- exp table preload (runs during the input DMAs) ----
            dmy = consts.tile([1, 8], f32, tag="dmy")
            nc.vector.memset(dmy[:], 0.0)
            dmy2 = consts.tile([1, 8], f32, tag="dmy2")
            nc.scalar.activation(dmy2[:], dmy[:],
                                 mybir.ActivationFunctionType.Exp)

            # ---- input DMAs: q/k weights first, then chunked xt ----
            wq_sb = consts.tile([128, CO, DL], bf16, tag="wq")
            wk_sb = consts.tile([128, CO, DL], bf16, tag="wk")
            wv_sb = consts.tile([128, CO, DL], bf16, tag="wv")
            nc.sync.dma_start(wq_sb[:], wqt_d.rearrange("(o p) f -> p o f", p=128))
            nc.sync.dma_start(wk_sb[:], wkt_d.rearrange("(o p) f -> p o f", p=128))
            bq_sb = consts.tile([128, MO], f32, tag="bq")
            bk_sb = consts.tile([128, MO], f32, tag="bk")
            nc.sync.dma_start(bq_sb[:], bq_d[:])
            nc.sync.dma_start(bk_sb[:], bk_d[:])

            xt_sb = xtp.tile([128, CO, N], bf16, tag="xt")
            xt_r = xt_d.rearrange("(o p) n -> p o n", p=128)
            for kc in range(CO):
                nc.sync.dma_start(xt_sb[:, kc, :], xt_r[:, kc, :])

            nc.sync.dma_start(wv_sb[:], wvt_d.rearrange("(o p) f -> p o f", p=128))
            bv_sb = consts.tile([1, DL], f32r, tag="bv")
            nc.sync.dma_start(bv_sb[:], bv_d[:])
            pw_sb = consts.tile([128, MO, C], f32r, tag="pw")
            nc.sync.dma_start(pw_sb[:], pwt_d.rearrange("(o p) f -> p o f", p=128))

            ones_f = consts.tile([1, 128], f32, tag="onesf")
            nc.vector.memset(ones_f[:], 1.0)
            ones_sb = consts.tile([1, 128], f32r, tag="ones")
            nc.vector.tensor_copy(ones_sb[:], ones_f[:])

            # ---- resident activations ----
            qt_sb = acts.tile([128, MO, N], f32r, tag="qt")    # [DL, N]
            kt_sb = acts.tile([128, MO, N], f32r, tag="kt")
            qt2_sb = acts.tile([128, MO, N], f32r, tag="qt2")  # halves swapped
            kt2_sb = acts.tile([128, MO, N], f32r, tag="kt2")
            v_sb = acts.tile([128, MCHUNKS, HL, D + 1], f32r, tag="v")
            yt_sb = acts.tile([128, MO, N], f32r, tag="yt")

            ones_col = consts.tile([128, 1], f32, tag="onescol")
            nc.vector.memset(ones_col[:], 1.0)
            nc.vector.tensor_copy(
                v_sb[:, :, :, D:],
                ones_col[:].to_broadcast([128, MCHUNKS, HL, 1]))

            # ---- phase-0 helpers ----
            def emit_qk_mms(w_sb, mo, nh, kcs):
                """8(or fewer) kc x 2 half matmuls into a psS tile; returns it."""
                ps = psS.tile([128, 1024], f32, tag="pss",
                              name=f"qk_{id(w_sb)}_{mo}_{nh}_{kcs[0]}")
                for kc in kcs:
                    for half in range(2):
                        nc.tensor.matmul(
                            ps[:, half * 512:(half + 1) * 512],
                            lhsT=w_sb[:, kc, mo * 128:(mo + 1) * 128],
                            rhs=xt_sb[:, kc,
                                      nh * 1024 + half * 512:
                                      nh * 1024 + (half + 1) * 512],
                            start=(kc == 0), stop=(kc == CO - 1),
                        )
                return ps

            def emit_qk_bias(ps, b_sb, o_sb, mo, nh):
                nsl0 = slice(nh * 1024, (nh + 1) * 1024)
                nc.vector.tensor_scalar_add(
                    o_sb[:, mo, nsl0], ps[:], b_sb[:, mo:mo + 1])

            def emit_qk_swap(o_sb, o2_sb, mo, nh):
                nsl0 = slice(nh * 1024, (nh + 1) * 1024)
                nc.vector.tensor_copy(o2_sb[0:64, mo, nsl0],
                                      o_sb[64:128, mo, nsl0])
                nc.vector.tensor_copy(o2_sb[64:128, mo, nsl0],
                                      o_sb[0:64, mo, nsl0])

            # split-burst state for mo1 tiles emitted inside the head loops
            qk_partial = {}

            def emit_qk_piece(which, mo, nh, part):
                """part 0: kc 0-3; part 1: kc 4-7 + bias; part 2: swap."""
                w_sb, b_sb, o_sb, o2_sb = (
                    (wq_sb, bq_sb, qt_sb, qt2_sb) if which == "q"
                    else (wk_sb, bk_sb, kt_sb, kt2_sb))
                key = (which, mo, nh)
                if part == 0:
                    ps = psS.tile([128, 1024], f32, tag="pss",
                                  name=f"qkp_{which}_{mo}_{nh}")
                    for kc in range(4):
                        for half in range(2):
                            nc.tensor.matmul(
                                ps[:, half * 512:(half + 1) * 512],
                                lhsT=w_sb[:, kc, mo * 128:(mo + 1) * 128],
                                rhs=xt_sb[:, kc,
                                          nh * 1024 + half * 512:
                                          nh * 1024 + (half + 1) * 512],
                                start=(kc == 0), stop=False,
                            )
                    qk_partial[key] = ps
                elif part == 1:
                    ps = qk_partial.pop(key)
                    for kc in range(4, 8):
                        for half in range(2):
                            nc.tensor.matmul(
                                ps[:, half * 512:(half + 1) * 512],
                                lhsT=w_sb[:, kc, mo * 128:(mo + 1) * 128],
                                rhs=xt_sb[:, kc,
                                          nh * 1024 + half * 512:
                                          nh * 1024 + (half + 1) * 512],
                                start=False, stop=(kc == CO - 1),
                            )
                    emit_qk_bias(ps, b_sb, o_sb, mo, nh)
                else:
                    emit_qk_swap(o_sb, o2_sb, mo, nh)

            # ---- lead-in: QK for mo0 (heads 0,1), kc-inner racing the DMA.
            # nh0 tiles first (they gate the first exp), primaries before the
            # swapped copies (SBUF->SBUF, off the psum ring) ----
            ps_q0 = emit_qk_mms(wq_sb, 0, 0, list(range(CO)))
            ps_k0 = emit_qk_mms(wk_sb, 0, 0, list(range(CO)))
            ps_q1 = emit_qk_mms(wq_sb, 0, 1, list(range(CO)))
            ps_k1 = emit_qk_mms(wk_sb, 0, 1, list(range(CO)))
            emit_qk_bias(ps_q0, bq_sb, qt_sb, 0, 0)
            emit_qk_bias(ps_k0, bk_sb, kt_sb, 0, 0)
            emit_qk_bias(ps_q1, bq_sb, qt_sb, 0, 1)
            emit_qk_bias(ps_k1, bk_sb, kt_sb, 0, 1)
            emit_qk_swap(qt_sb, qt2_sb, 0, 0)
            emit_qk_swap(kt_sb, kt2_sb, 0, 0)
            emit_qk_swap(qt_sb, qt2_sb, 0, 1)
            emit_qk_swap(kt_sb, kt2_sb, 0, 1)

            # ---- V chunk (emitted inside head 0's loop) ----
            def emit_v_chunk(i):
                ps = psS.tile([128, HL, D], f32, tag="pss", name=f"v{i}")
                for kc in range(CO):
                    nc.tensor.matmul(
                        ps[:],
                        lhsT=xt_sb[:, kc, i * 128:(i + 1) * 128],
                        rhs=wv_sb[:, kc, :],
                        start=(kc == 0), stop=False,
                    )
                nc.tensor.matmul(
                    ps[:], lhsT=ones_sb[:], rhs=bv_sb[:],
                    start=False, stop=True,
                )
                nc.vector.tensor_copy(v_sb[:, i, :, :D], ps[:])

            # ---- per-head norm: free the psB accumulators fast, then
            # reciprocal + GpSimd partition-broadcast + mul off-band ----
            def emit_norm(hn, psBs_n):
                mo_n = hn // 2
                pb_n = 64 * (hn % 2)
                obs = []
                for nb in range(4):
                    ob = small.tile([D + 1, 512], f32, tag="ob",
                                    name=f"ob{hn}_{nb}")
                    nc.vector.tensor_copy(ob[:], psBs_n[nb][:])
                    obs.append(ob)
                for nb in range(4):
                    nsl = slice(nb * 512, (nb + 1) * 512)
                    rc = small.tile([1, 512], f32, tag="rc",
                                    name=f"rc{hn}_{nb}")
                    nc.vector.reciprocal_approx_fast(rc[:],
                                                     obs[nb][D:D + 1, :])
                    bc = small.tile([D, 512], f32, tag="bc",
                                    name=f"bc{hn}_{nb}")
                    nc.gpsimd.partition_broadcast(bc[:], rc[:])
                    nc.vector.tensor_mul(
                        yt_sb[pb_n:pb_n + D, mo_n, nsl], obs[nb][:D, :], bc[:])

            # ---- attention head loop ----
            psBs_by_h = {}
            pending = []     # queue of (h, i, ei) awaiting B matmuls

            def emit_B(hb, ib, eib):
                if ib == 0:
                    if hb > 0:
                        emit_norm(hb - 1, psBs_by_h.pop(hb - 1))
                    psBs_by_h[hb] = [
                        psB.tile([D + 1, 512], f32, tag="psb",
                                 name=f"psb_{hb}_{nb}")
                        for nb in range(4)]
                for nb in range(4):
                    nc.tensor.matmul(
                        psBs_by_h[hb][nb][:],
                        lhsT=v_sb[:, ib, hb, :],
                        rhs=eib[:, nb * 512:(nb + 1) * 512],
                        start=(ib == 0), stop=(ib == MCHUNKS - 1),
                    )

            # mo1 QK pieces scheduled into heads 1-2 at (head, chunk):
            filler = {
                (1, 0): ("q", 1, 0, 0), (1, 1): ("q", 1, 0, 1),
                (1, 2): ("q", 1, 0, 2),
                (1, 4): ("q", 1, 1, 0), (1, 5): ("q", 1, 1, 1),
                (1, 6): ("q", 1, 1, 2),
                (1, 8): ("k", 1, 0, 0), (1, 9): ("k", 1, 0, 1),
                (1, 10): ("k", 1, 0, 2),
                (2, 0): ("k", 1, 1, 0), (2, 1): ("k", 1, 1, 1),
                (2, 2): ("k", 1, 1, 2),
            }

            for h in range(HL):
                mo = h // 2
                pb = 64 * (h % 2)
                for i in range(MCHUNKS):
                    # A-matmul row-group alternation: chunks 0,1 primary,
                    # then swapped on even chunks (all adjacent pairs but
                    # (0,1) land on disjoint PE row groups)
                    if i >= 2 and i % 2 == 0:
                        kts, qts, pbi = kt2_sb, qt2_sb, pb ^ 64
                    else:
                        kts, qts, pbi = kt_sb, qt_sb, pb
                    ei = ei_pool.tile([128, N], f32r, tag="ei")
                    for nh in range(2):
                        ps = psS.tile([128, 1024], f32, tag="pss",
                                      name=f"a_{h}_{i}_{nh}")
                        for half in range(2):
                            nc.tensor.matmul(
                                ps[:, half * 512:(half + 1) * 512],
                                lhsT=kts[pbi:pbi + D, mo,
                                         i * 128:(i + 1) * 128],
                                rhs=qts[pbi:pbi + D, mo,
                                        nh * 1024 + half * 512:
                                        nh * 1024 + (half + 1) * 512],
                                start=True, stop=True,
                            )
                        nc.scalar.activation(
                            ei[:, nh * 1024:(nh + 1) * 1024], ps[:],
                            mybir.ActivationFunctionType.Exp,
                        )
                    if h == 0:
                        emit_v_chunk(i)
                    elif (h, i) in filler:
                        emit_qk_piece(*filler[(h, i)])
                    if len(pending) >= 2:
                        emit_B(*pending.pop(0))
                    pending.append((h, i, ei))
            for p in pending:
                emit_B(*p)
            emit_norm(HL - 1, psBs_by_h.pop(HL - 1))

            # ---- phase D: P^T = pwT.T @ Y^T, tail; bf16 out ----
            for nbp in range(2):
                for cc in range(CO):
                    ps = psS.tile([128, 1024], f32, tag="pss",
                                  name=f"d_{nbp}_{cc}")
                    for j in range(2):
                        nb = 2 * nbp + j
                        for jc in range(MO):
                            nc.tensor.matmul(
                                ps[:, j * 512:(j + 1) * 512],
                                lhsT=pw_sb[:, jc, cc * 128:(cc + 1) * 128],
                                rhs=yt_sb[:, jc, nb * 512:(nb + 1) * 512],
                                start=(jc == 0), stop=(jc == MO - 1),
                            )
                    st = stp.tile([128, 1024], bf16, tag="st")
                    nc.vector.tensor_copy(st[:, :512], ps[:, :512])
                    nc.scalar.copy(st[:, 512:], ps[:, 512:])
                    nc.sync.dma_start(
                        out_d[cc * 128:(cc + 1) * 128,
                              nbp * 1024:(nbp + 1) * 1024], st[:])

    nc.compile()
    return nc


def shard_inputs(x, qkv_w, qkv_b, proj_w):
    """Build the 8 per-core input maps (host-side sharding)."""
    in_maps = []
    for core in range(N_CORES):
        b, g = divmod(core, G)
        gs = slice(g * DL, (g + 1) * DL)
        xt = np.ascontiguousarray(x[b].T)
        wq = qkv_w[0 * C:1 * C][gs] * SCALE     # fold 1/sqrt(D) into Q
        wk = qkv_w[1 * C:2 * C][gs]
        wv = qkv_w[2 * C:3 * C][gs]
        in_maps.append({
            "xt": np.ascontiguousarray(xt).astype(ml_dtypes.bfloat16),
            "wqt": np.ascontiguousarray(wq.T).astype(ml_dtypes.bfloat16),
            "wkt": np.ascontiguousarray(wk.T).astype(ml_dtypes.bfloat16),
            "wvt": np.ascontiguousarray(wv.T).astype(ml_dtypes.bfloat16),
            "bq": np.ascontiguousarray(
                (qkv_b[0 * C:1 * C][gs] * SCALE).reshape(DL // 128, 128).T),
            "bk": np.ascontiguousarray(
                qkv_b[1 * C:2 * C][gs].reshape(DL // 128, 128).T),
            "bv": np.ascontiguousarray(qkv_b[2 * C:3 * C][gs].reshape(1, DL)),
            "pwt": np.ascontiguousarray(proj_w[:, gs].T),
        })
    return in_maps


def unshard_output(results, proj_b):
    """results: list of 8 dicts with 'out' [C, N] bf16 partial projections."""
    out = np.empty((B, N, C), dtype=np.float32)
    for b in range(B):
        acc = results[b * G]["out"].astype(np.float32)
        for g in range(1, G):
            acc = acc + results[b * G + g]["out"].astype(np.float32)
        out[b] = acc.T + proj_b
    return out


def kernel(x, qkv_w, qkv_b, proj_w, proj_b):
    from concourse.bass_utils import run_bass_kernel_spmd

    x = np.asarray(x, dtype=np.float32)
    qkv_w = np.asarray(qkv_w, dtype=np.float32)
    qkv_b = np.asarray(qkv_b, dtype=np.float32)
    proj_w = np.asarray(proj_w, dtype=np.float32)
    proj_b = np.asarray(proj_b, dtype=np.float32)

    if "nc" not in _CACHE:
        _CACHE["nc"] = build_kernel()
    nc = _CACHE["nc"]

    in_maps = shard_inputs(x, qkv_w, qkv_b, proj_w)
    res = run_bass_kernel_spmd(nc, in_maps, list(range(N_CORES)))
    return unshard_output(res.results, proj_b)


# revision 8
# speedup vs baseline: 1.3723x; 1.3096x over previous
"""Multi-head attention (B=2, N=2048, C=1024, H=16, D=64) on 8 TRN2 NeuronCores.

Sharding: core = b*4 + g  (b in {0,1} data parallel over batch,
g in {0..3} tensor parallel over head groups of HL=4 heads).

v3 schedule, built around two measured hardware facts: (1) the ScalarE exp
stream (~163us) is a hard per-core floor, and (2) each matmul costs
LDWEIGHTS + N/2.4GHz serialized (~380ns at N=512) unless adjacent matmuls
sit on disjoint PE row groups, in which case they overlap.

  - A-phase (S^T = K^T.T @ Q^T, K=64): the nh0 half reads the primary
    qt/kt (rows pb..pb+64) and the nh1 half reads the partition-swapped
    copies qt2/kt2 (rows pb^64..), so the four matmuls of every chunk
    alternate PE row groups and run pairwise-concurrent.
  - everything is bf16 on the PE (fast weight load; psum accumulate f32).
  - lead-in: chunked-xt DMA races the QK-mo0 matmuls, first exp ~25us.
  - V is computed inside head 0's loop (no bias matmuls: bias comes from a
    GpSimd partition_broadcast of bv + a tensor_tensor add on the copy out
    of psum); QK-mo1 is split into 2-kc pieces across heads 1-2.
  - per-head norm: one [65,512] copy frees each PSUM accumulator, then
    reciprocal + GpSimd partition_broadcast + DVE mul off-band.
  - projection tail: f32->bf16 casts split across DVE+ScalarE, bf16 DMA.
Host: out[b] = sum_g P^T[b,g].T + proj_b  (bf16 partials summed in f32).
"""

import numpy as np
import ml_dtypes

B, N, C = 2, 2048, 1024
H = 16
D = C // H          # 64
G = 4               # head groups (tensor parallel)
HL = H // G         # 4 heads per core
DL = HL * D         # 256 local head dims
N_CORES = 8
SCALE = 1.0 / np.sqrt(np.float32(D))

MCHUNKS = N // 128  # 16
CO = C // 128       # 8 chunks of the contraction dim c
MO = DL // 128      # 2 chunks of the local head dims

_CACHE = {}


def build_kernel():
    import concourse.bass as bass
    import concourse.mybir as mybir
    import concourse.tile as tile
    from concourse import bacc

    f32 = mybir.dt.float32
    bf16 = mybir.dt.bfloat16

    nc = bacc.Bacc("TRN2", target_bir_lowering=False, debug=False,
                   num_devices=N_CORES)

    xt_d = nc.dram_tensor("xt", [C, N], bf16, kind="ExternalInput").ap()
    wqt_d = nc.dram_tensor("wqt", [C, DL], bf16, kind="ExternalInput").ap()
    wkt_d = nc.dram_tensor("wkt", [C, DL], bf16, kind="ExternalInput").ap()
    wvt_d = nc.dram_tensor("wvt", [C, DL], bf16, kind="ExternalInput").ap()
    bq_d = nc.dram_tensor("bq", [128, MO], f32, kind="ExternalInput").ap()
    bk_d = nc.dram_tensor("bk", [128, MO], f32, kind="ExternalInput").ap()
    bv_d = nc.dram_tensor("bv", [1, DL], f32, kind="ExternalInput").ap()
    pwt_d = nc.dram_tensor("pwt", [DL, C], bf16, kind="ExternalInput").ap()
    out_d = nc.dram_tensor("out", [C, N], bf16, kind="ExternalOutput").ap()

    with tile.TileContext(nc) as tc:
        with (
            tc.tile_pool(name="consts", bufs=1) as consts,
            tc.tile_pool(name="acts", bufs=1) as acts,
            tc.tile_pool(name="xtp", bufs=1) as xtp,
            tc.tile_pool(name="small", bufs=4) as small,
            tc.tile_pool(name="stp", bufs=3) as stp,
            tc.tile_pool(name="eip", bufs=4) as ei_pool,
            tc.tile_pool(name="psS", bufs=2, space="PSUM") as psS,
            tc.tile_pool(name="psB", bufs=4, space="PSUM") as psB,
        ):
            # ---- exp table preload (runs during the input DMAs) ----
            dmy = consts.tile([1, 8], f32, tag="dmy")
            nc.vector.memset(dmy[:], 0.0)
            dmy2 = consts.tile([1, 8], f32, tag="dmy2")
            nc.scalar.activation(dmy2[:], dmy[:],
                                 mybir.ActivationFunctionType.Exp)

            # ---- input DMAs: q/k weights first, then chunked xt ----
            wq_sb = consts.tile([128, CO, DL], bf16, tag="wq")
            wk_sb = consts.tile([128, CO, DL], bf16, tag="wk")
            wv_sb = consts.tile([128, CO, DL], bf16, tag="wv")
            nc.sync.dma_start(wq_sb[:], wqt_d.rearrange("(o p) f -> p o f", p=128))
            nc.sync.dma_start(wk_sb[:], wkt_d.rearrange("(o p) f -> p o f", p=128))
            bq_sb = consts.tile([128, MO], f32, tag="bq")
            bk_sb = consts.tile([128, MO], f32, tag="bk")
            nc.sync.dma_start(bq_sb[:], bq_d[:])
            nc.sync.dma_start(bk_sb[:], bk_d[:])

            xt_sb = xtp.tile([128, CO, N], bf16, tag="xt")
            xt_r = xt_d.rearrange("(o p) n -> p o n", p=128)
            for kc in range(CO):
                nc.sync.dma_start(xt_sb[:, kc, :], xt_r[:, kc, :])

            nc.sync.dma_start(wv_sb[:], wvt_d.rearrange("(o p) f -> p o f", p=128))
            bv_sb = consts.tile([1, DL], f32, tag="bv")
            nc.sync.dma_start(bv_sb[:], bv_d[:])
            pw_sb = consts.tile([128, MO, C], bf16, tag="pw")
            nc.sync.dma_start(pw_sb[:], pwt_d.rearrange("(o p) f -> p o f", p=128))

            # ---- resident activations ----
            qt_sb = acts.tile([128, MO, N], bf16, tag="qt")    # [DL, N]
            kt_sb = acts.tile([128, MO, N], bf16, tag="kt")
            qt2_sb = acts.tile([128, MO, N], bf16, tag="qt2")  # halves swapped
            kt2_sb = acts.tile([128, MO, N], bf16, tag="kt2")
            v_sb = acts.tile([128, MCHUNKS, HL, D + 1], bf16, tag="v")
            yt_sb = acts.tile([128, MO, N], bf16, tag="yt")

            ones_col = consts.tile([128, 1], f32, tag="onescol")
            nc.vector.memset(ones_col[:], 1.0)
            nc.vector.tensor_copy(
                v_sb[:, :, :, D:],
                ones_col[:].to_broadcast([128, MCHUNKS, HL, 1]))
            # broadcast V bias to all partitions once (GpSimd)
            bvb_sb = consts.tile([128, HL, D], f32, tag="bvb")
            nc.gpsimd.partition_broadcast(bvb_sb[:], bv_sb[:])

            # ---- phase-0 helpers ----
            def emit_qk_mms(w_sb, mo, nh, kc_lo, kc_hi, ps=None):
                if ps is None:
                    ps = psS.tile([128, 1024], f32, tag="pss",
                                  name=f"qk{id(w_sb) % 97}_{mo}_{nh}")
                for kc in range(kc_lo, kc_hi):
                    for half in range(2):
                        nc.tensor.matmul(
                            ps[:, half * 512:(half + 1) * 512],
                            lhsT=w_sb[:, kc, mo * 128:(mo + 1) * 128],
                            rhs=xt_sb[:, kc,
                                      nh * 1024 + half * 512:
                                      nh * 1024 + (half + 1) * 512],
                            start=(kc == 0), stop=(kc == CO - 1),
                        )
                return ps

            def emit_qk_bias(ps, b_sb, o_sb, mo, nh):
                nsl0 = slice(nh * 1024, (nh + 1) * 1024)
                nc.vector.tensor_scalar_add(
                    o_sb[:, mo, nsl0], ps[:], b_sb[:, mo:mo + 1])

            def emit_qk_swap(o_sb, o2_sb, mo, nh):
                nsl0 = slice(nh * 1024, (nh + 1) * 1024)
                nc.vector.tensor_copy(o2_sb[0:64, mo, nsl0],
                                      o_sb[64:128, mo, nsl0])
                nc.vector.tensor_copy(o2_sb[64:128, mo, nsl0],
                                      o_sb[0:64, mo, nsl0])

            # mo1 tiles interleaved into heads 1-2 as single-chunk bursts
            # (a psum tile held across chunks would starve the 2-slot ring)
            def emit_qk_tile_full(which, nh):
                w_sb, b_sb, o_sb, o2_sb = (
                    (wq_sb, bq_sb, qt_sb, qt2_sb) if which == "q"
                    else (wk_sb, bk_sb, kt_sb, kt2_sb))
                ps = emit_qk_mms(w_sb, 1, nh, 0, CO)
                emit_qk_bias(ps, b_sb, o_sb, 1, nh)
                emit_qk_swap(o_sb, o2_sb, 1, nh)

            # ---- A-phase chunk: nh0 on primary rows, nh1 on swapped rows
            # (disjoint PE row groups -> the 4 matmuls run pairwise) ----
            def emit_A(h, i, ei):
                mo = h // 2
                pb = 64 * (h % 2)
                pc = pb ^ 64
                ps0 = psS.tile([128, 1024], f32, tag="pss", name=f"a{h}_{i}_0")
                ps1 = psS.tile([128, 1024], f32, tag="pss", name=f"a{h}_{i}_1")
                for half in range(2):
                    nc.tensor.matmul(
                        ps0[:, half * 512:(half + 1) * 512],
                        lhsT=kt_sb[pb:pb + D, mo, i * 128:(i + 1) * 128],
                        rhs=qt_sb[pb:pb + D, mo,
                                  half * 512:half * 512 + 512],
                        start=True, stop=True,
                    )
                    nc.tensor.matmul(
                        ps1[:, half * 512:(half + 1) * 512],
                        lhsT=kt2_sb[pc:pc + D, mo, i * 128:(i + 1) * 128],
                        rhs=qt2_sb[pc:pc + D, mo,
                                   1024 + half * 512:1024 + half * 512 + 512],
                        start=True, stop=True,
                    )
                nc.scalar.activation(ei[:, 0:1024], ps0[:],
                                     mybir.ActivationFunctionType.Exp)
                nc.scalar.activation(ei[:, 1024:2048], ps1[:],
                                     mybir.ActivationFunctionType.Exp)

            # ---- V chunk (inside head 0's loop; bias via bvb add) ----
            def emit_v_chunk(i):
                ps = psS.tile([128, HL, D], f32, tag="pss", name=f"v{i}")
                for kc in range(CO):
                    nc.tensor.matmul(
                        ps[:],
                        lhsT=xt_sb[:, kc, i * 128:(i + 1) * 128],
                        rhs=wv_sb[:, kc, :],
                        start=(kc == 0), stop=(kc == CO - 1),
                    )
                nc.vector.tensor_add(v_sb[:, i, :, :D], ps[:], bvb_sb[:])

            # ---- per-head norm ----
            def emit_norm(hn, psBs_n):
                mo_n = hn // 2
                pb_n = 64 * (hn % 2)
                obs = []
                for nb in range(4):
                    ob = small.tile([D + 1, 512], f32, tag="ob",
                                    name=f"ob{hn}_{nb}")
                    nc.vector.tensor_copy(ob[:], psBs_n[nb][:])
                    obs.append(ob)
                for nb in range(4):
                    nsl = slice(nb * 512, (nb + 1) * 512)
                    rc = small.tile([1, 512], f32, tag="rc",
                                    name=f"rc{hn}_{nb}")
                    nc.vector.reciprocal_approx_fast(rc[:],
                                                     obs[nb][D:D + 1, :])
                    bc = small.tile([D, 512], f32, tag="bc",
                                    name=f"bc{hn}_{nb}")
                    nc.gpsimd.partition_broadcast(bc[:], rc[:])
                    nc.vector.tensor_mul(
                        yt_sb[pb_n:pb_n + D, mo_n, nsl], obs[nb][:D, :], bc[:])

            # ---- lead-in: QK-mo0 with the chunk-0 exp pulled as early as
            # possible (q-nh0 + k-nh0 -> A(0,0) nh0 -> exp; then nh1) ----
            ps_q0 = emit_qk_mms(wq_sb, 0, 0, 0, CO)
            ps_k0 = emit_qk_mms(wk_sb, 0, 0, 0, CO)
            emit_qk_bias(ps_q0, bq_sb, qt_sb, 0, 0)
            emit_qk_bias(ps_k0, bk_sb, kt_sb, 0, 0)
            emit_qk_swap(qt_sb, qt2_sb, 0, 0)
            emit_qk_swap(kt_sb, kt2_sb, 0, 0)
            ps_q1 = emit_qk_mms(wq_sb, 0, 1, 0, CO)
            ps_k1 = emit_qk_mms(wk_sb, 0, 1, 0, CO)
            emit_qk_bias(ps_q1, bq_sb, qt_sb, 0, 1)
            emit_qk_bias(ps_k1, bk_sb, kt_sb, 0, 1)
            emit_qk_swap(qt_sb, qt2_sb, 0, 1)
            emit_qk_swap(kt_sb, kt2_sb, 0, 1)

            # ---- attention head loop ----
            psBs_by_h = {}
            pending = []     # queue of (h, i, ei) awaiting B matmuls

            def emit_B(hb, ib, eib):
                if ib == 0:
                    if hb > 0:
                        emit_norm(hb - 1, psBs_by_h.pop(hb - 1))
                    psBs_by_h[hb] = [
                        psB.tile([D + 1, 512], f32, tag="psb",
                                 name=f"psb_{hb}_{nb}")
                        for nb in range(4)]
                for nb in range(4):
                    nc.tensor.matmul(
                        psBs_by_h[hb][nb][:],
                        lhsT=v_sb[:, ib, hb, :],
                        rhs=eib[:, nb * 512:(nb + 1) * 512],
                        start=(ib == 0), stop=(ib == MCHUNKS - 1),
                    )

            # mo1 QK tile bursts at (head, chunk): q/k-nh0 + q-nh1 in head 1,
            # k-nh1 in head 2 (first needed at head 2 chunk 8)
            filler = {
                (1, 0): ("q", 0), (1, 5): ("k", 0), (1, 10): ("q", 1),
                (2, 0): ("k", 1),
            }

            for h in range(HL):
                for i in range(MCHUNKS):
                    ei = ei_pool.tile([128, N], bf16, tag="ei")
                    emit_A(h, i, ei)
                    if h == 0:
                        emit_v_chunk(i)
                    elif (h, i) in filler:
                        emit_qk_tile_full(*filler[(h, i)])
                    if len(pending) >= 2:
                        emit_B(*pending.pop(0))
                    pending.append((h, i, ei))
            for p in pending:
                emit_B(*p)
            emit_norm(HL - 1, psBs_by_h.pop(HL - 1))

            # ---- phase D: P^T = pwT.T @ Y^T, tail; bf16 out ----
            for nbp in range(2):
                for cc in range(CO):
                    ps = psS.tile([128, 1024], f32, tag="pss",
                                  name=f"d_{nbp}_{cc}")
                    for j in range(2):
                        nb = 2 * nbp + j
                        for jc in range(MO):
                            nc.tensor.matmul(
                                ps[:, j * 512:(j + 1) * 512],
                                lhsT=pw_sb[:, jc, cc * 128:(cc + 1) * 128],
                                rhs=yt_sb[:, jc, nb * 512:(nb + 1) * 512],
                                start=(jc == 0), stop=(jc == MO - 1),
                            )
                    st = stp.tile([128, 1024], bf16, tag="st")
                    nc.vector.tensor_copy(st[:, :512], ps[:, :512])
                    nc.scalar.copy(st[:, 512:], ps[:, 512:])
                    nc.sync.dma_start(
                        out_d[cc * 128:(cc + 1) * 128,
                              nbp * 1024:(nbp + 1) * 1024], st[:])

    nc.compile()
    return nc


def shard_inputs(x, qkv_w, qkv_b, proj_w):
    """Build the 8 per-core input maps (host-side sharding)."""
    in_maps = []
    for core in range(N_CORES):
        b, g = divmod(core, G)
        gs = slice(g * DL, (g + 1) * DL)
        xt = np.ascontiguousarray(x[b].T)
        wq = qkv_w[0 * C:1 * C][gs] * SCALE     # fold 1/sqrt(D) into Q
        wk = qkv_w[1 * C:2 * C][gs]
        wv = qkv_w[2 * C:3 * C][gs]
        in_maps.append({
            "xt": np.ascontiguousarray(xt).astype(ml_dtypes.bfloat16),
            "wqt": np.ascontiguousarray(wq.T).astype(ml_dtypes.bfloat16),
            "wkt": np.ascontiguousarray(wk.T).astype(ml_dtypes.bfloat16),
            "wvt": np.ascontiguousarray(wv.T).astype(ml_dtypes.bfloat16),
            "bq": np.ascontiguousarray(
                (qkv_b[0 * C:1 * C][gs] * SCALE).reshape(DL // 128, 128).T),
            "bk": np.ascontiguousarray(
                qkv_b[1 * C:2 * C][gs].reshape(DL // 128, 128).T),
            "bv": np.ascontiguousarray(qkv_b[2 * C:3 * C][gs].reshape(1, DL)),
            "pwt": np.ascontiguousarray(proj_w[:, gs].T).astype(
                ml_dtypes.bfloat16),
        })
    return in_maps


def unshard_output(results, proj_b):
    """results: list of 8 dicts with 'out' [C, N] bf16 partial projections."""
    out = np.empty((B, N, C), dtype=np.float32)
    for b in range(B):
        acc = results[b * G]["out"].astype(np.float32)
        for g in range(1, G):
            acc = acc + results[b * G + g]["out"].astype(np.float32)
        out[b] = acc.T + proj_b
    return out


def kernel(x, qkv_w, qkv_b, proj_w, proj_b):
    from concourse.bass_utils import run_bass_kernel_spmd

    x = np.asarray(x, dtype=np.float32)
    qkv_w = np.asarray(qkv_w, dtype=np.float32)
    qkv_b = np.asarray(qkv_b, dtype=np.float32)
    proj_w = np.asarray(proj_w, dtype=np.float32)
    proj_b = np.asarray(proj_b, dtype=np.float32)

    if "nc" not in _CACHE:
        _CACHE["nc"] = build_kernel()
    nc = _CACHE["nc"]

    in_maps = shard_inputs(x, qkv_w, qkv_b, proj_w)
    res = run_bass_kernel_spmd(nc, in_maps, list(range(N_CORES)))
    return unshard_output(res.results, proj_b)
